# revision 22
# baseline (speedup 1.0000x reference)
"""Causal single-head attention forward (B=4, T=4096, C=256, H=64) on 8 NeuronCores.

Sharding: core = (batch, kv_parity).  Each core processes ALL queries of its
batch but only kv tiles (128 keys) whose global tile index has its parity
(even/odd interleave), which balances the causal workload across the two
cores of a batch.  Each core emits unnormalized numerator+denominator stacked
as ud[65, T] (rows 0:64 = (exp(S)@V)^T, row 64 = sum exp(S)); the host
merges: out = (u0+u1)/(d0+d1), transposed back.

The compiled program is parity-uniform; parity only enters through host-
prepared data (gathered xkv columns and the diagonal-pair mask values).

Engine plan per core:
- PE (bf16): q projections from full x, k/v projections from the gathered
  parity columns, S^T = K Q^T per 128-key tile, AV with V stationary (ones
  column folds the denominator).  The diagonal pair's second tile is
  col-trimmed to [256, 512) (parity-uniform superset of the causal region).
  AV lags the QK/exp stream by 2 pairs so the PE queue never drains (keeps
  the tensor engine's p-state ramped).
- ACT: exact exp (fp32 psum -> bf16) for the diagonal pair + even non-diag
  pairs; table pre-warmed during input DMA.
- DVE: exp for odd non-diag pairs via one tensor_scalar emitting bf16 BITS
  as int16 (Schraudolph: bits = round(s*A + B), ~+-3% per-element, max-norm
  safe), q/k psum->bf16 casts, diagonal mask multiplies.
- Pool (gpsimd): v psum->bf16 copies, ones-column memsets, av psum->sbuf
  output copies.
"""

import sys

for _p in ("/opt/trn_rl_repo", "/root/.axon_site/_ro/trn_rl_repo"):
    if _p not in sys.path:
        sys.path.append(_p)

from contextlib import ExitStack

import numpy as np

import concourse.bacc as bacc
import concourse.bass as bass
import concourse.tile as tile
from concourse import mybir
from concourse.bass_utils import run_bass_kernel_spmd

B, T, C, H = 4, 4096, 256, 64
QB = 512         # query block width
NQB = T // QB    # 8 query blocks
KT = 128         # kv tile width
TK = T // 2      # gathered kv columns per core
F32 = mybir.dt.float32
BF16 = mybir.dt.bfloat16
FP8 = mybir.dt.float8e4
I16 = mybir.dt.int16
I8 = mybir.dt.int8
DR = mybir.MatmulPerfMode.DoubleRow
SCALE = float(C) ** -0.5
# Schraudolph: bits = round(s*A + B) ~ exp(s*SCALE), emitted as raw bf16/fp8 bits
SCH_A = 128.0 / float(np.log(2.0)) * SCALE
SCH_B = 16248.65
SCH_A8 = 8.0 / float(np.log(2.0)) * SCALE
SCH_B8 = 55.55
O1 = 256         # uniform col-trim offset for the diagonal pair's 2nd tile

_NC = None


def build_nc() -> bass.Bass:
    nc = bacc.Bacc("TRN2", target_bir_lowering=False, debug=False)
    xq = nc.declare_dram_parameter("xq", [128, 2, T], BF16, isOutput=False)
    xkv = nc.declare_dram_parameter("xkv", [128, 2, TK], BF16, isOutput=False)
    wq = nc.declare_dram_parameter("wq", [128, 2, H], BF16, isOutput=False)
    wk = nc.declare_dram_parameter("wk", [128, 2, H], BF16, isOutput=False)
    wv = nc.declare_dram_parameter("wv", [128, 2, H], BF16, isOutput=False)
    msk = nc.declare_dram_parameter("msk", [KT, QB], BF16, isOutput=False)
    ud = nc.declare_dram_parameter("ud", [H + 1, T], F32, isOutput=True)

    with tile.TileContext(nc) as tc, ExitStack() as ctx:
        persist = ctx.enter_context(tc.tile_pool(name="persist", bufs=1))
        pexp = ctx.enter_context(tc.tile_pool(name="exp", bufs=4))
        pout = ctx.enter_context(tc.tile_pool(name="out", bufs=2))
        pproj = ctx.enter_context(tc.tile_pool(name="pproj", bufs=2, space="PSUM"))
        pqk = ctx.enter_context(tc.tile_pool(name="pqk", bufs=2, space="PSUM"))
        pav = ctx.enter_context(tc.tile_pool(name="pav", bufs=2, space="PSUM"))

        # ---- input DMAs: need-first chunk order, split across SP+ACT queues --
        xkv_sb = persist.tile([128, 2, TK], BF16, tag="xkv")
        xq_sb = persist.tile([128, 2, T], BF16, tag="xq")
        nc.sync.dma_start(out=xkv_sb[:, :, 0:QB], in_=xkv[:, :, 0:QB])
        nc.scalar.dma_start(out=xq_sb[:, :, 0:QB], in_=xq[:, :, 0:QB])
        w_sb = {}
        for name, dram in (("k", wk), ("v", wv), ("q", wq)):
            t = persist.tile([128, 2, H], BF16, tag=f"w{name}")
            nc.sync.dma_start(out=t[:], in_=dram[:])
            w_sb[name] = t
        m_sb = persist.tile([KT, QB], BF16, tag="mask")
        nc.sync.dma_start(out=m_sb[:], in_=msk[:])
        # warm the ACT exp table while DMAs stream
        warm = persist.tile([1, 2], F32, tag="warm")
        nc.vector.memset(warm[:], 0.0)
        nc.scalar.activation(warm[:], warm[:], mybir.ActivationFunctionType.Exp)
        # remaining x chunks, interleaved need-first
        for j in range(1, NQB):
            nc.scalar.dma_start(
                out=xq_sb[:, :, QB * j : QB * (j + 1)],
                in_=xq[:, :, QB * j : QB * (j + 1)],
            )
            if j < TK // QB:
                nc.sync.dma_start(
                    out=xkv_sb[:, :, QB * j : QB * (j + 1)],
                    in_=xkv[:, :, QB * j : QB * (j + 1)],
                )

        # ---- projections (bf16, contract C in 2 chunks) ---------------------
        q_sb = [None] * NQB           # bf16 [64, QB]
        k_sb = [None] * (TK // QB)    # bf16 [64, QB] local gathered layout
        v_sb = [None] * NQB           # bf16 [128, 2, 65] per pair (diag AV)
        v8_sb = [None] * NQB          # fp8 [128, 2, 65] per pair (DR AV)

        def proj_qk(which, src, j, dst_list):
            ps = pproj.tile([64, QB], F32, tag="proj")
            for c in range(2):
                nc.tensor.matmul(
                    ps[:], lhsT=w_sb[which][:, c, :],
                    rhs=src[:, c, QB * j : QB * (j + 1)],
                    start=(c == 0), stop=(c == 1),
                )
            t = persist.tile([64, QB], BF16, tag=f"{which}{j}")
            nc.vector.tensor_copy(t[:], ps[:])
            dst_list[j] = t

        def proj_v(P):
            # pair P covers local kv tiles 2P, 2P+1 -> gathered cols 128s
            ps = pproj.tile([128, 2, H], F32, tag="proj")
            for h in range(2):
                s = 2 * P + h
                for c in range(2):
                    nc.tensor.matmul(
                        ps[:, h, :],
                        lhsT=xkv_sb[:, c, KT * s : KT * (s + 1)],
                        rhs=w_sb["v"][:, c, :],
                        start=(c == 0), stop=(c == 1),
                    )
            t = persist.tile([128, 2, H + 1], BF16, tag=f"v{P}")
            nc.vector.tensor_copy(t[:, :, 0:H], ps[:])
            nc.gpsimd.memset(t[:, :, H : H + 1], 1.0)
            v_sb[P] = t
            t8 = persist.tile([128, 2, H + 16], FP8, tag=f"v8{P}")
            nc.vector.tensor_copy(t8[:, :, 0:H], ps[:])
            nc.gpsimd.memset(t8[:, :, H : H + 16], 1.0)
            v8_sb[P] = t8

        # k/v first (block 0's deps), then interleave q
        for j in range(TK // QB):
            proj_qk("k", xkv_sb, j, k_sb)
            proj_v(2 * j)
            proj_v(2 * j + 1)
            proj_qk("q", xq_sb, 2 * j, q_sb)
            proj_qk("q", xq_sb, 2 * j + 1, q_sb)

        # ---- attention -------------------------------------------------------
        def k_slice(s):  # local kv tile s -> gathered k columns
            return k_sb[s // 4][:, KT * (s % 4) : KT * (s % 4 + 1)]

        nslot = [0]

        def emit_qk_exp(p, P):
            diag = P == p
            if not diag:
                nslot[0] += 1
            qk2 = pqk.tile([KT, 2 * QB], F32, tag="qk")
            nc.tensor.matmul(
                qk2[:, 0:QB], lhsT=k_slice(2 * P), rhs=q_sb[p][:],
                start=True, stop=True,
            )
            if diag:
                nc.tensor.matmul(
                    qk2[:, QB + O1 : 2 * QB], lhsT=k_slice(2 * P + 1),
                    rhs=q_sb[p][:, O1:QB], start=True, stop=True,
                )
            else:
                nc.tensor.matmul(
                    qk2[:, QB : 2 * QB], lhsT=k_slice(2 * P + 1), rhs=q_sb[p][:],
                    start=True, stop=True,
                )
            if diag:
                ex = pexp.tile([KT, 2 * QB], BF16, tag="exp")
                nc.scalar.activation(
                    ex[:, 0:QB], qk2[:, 0:QB],
                    mybir.ActivationFunctionType.Exp, scale=SCALE,
                )
                nc.scalar.activation(
                    ex[:, QB + O1 : 2 * QB], qk2[:, QB + O1 : 2 * QB],
                    mybir.ActivationFunctionType.Exp, scale=SCALE,
                )
                # masks: region h0 = ex[:, 0:256] (*= msk[:, 0:256]),
                #        region h1 = ex[:, 768:1024] (*= msk[:, 256:512])
                nc.gpsimd.tensor_mul(ex[:, 0:O1], ex[:, 0:O1], m_sb[:, 0:O1])
                nc.gpsimd.tensor_mul(
                    ex[:, QB + O1 : 2 * QB], ex[:, QB + O1 : 2 * QB],
                    m_sb[:, O1:QB],
                )
                return ex
            ex = pexp.tile([KT, 2 * QB], FP8, tag="exp8")
            if nslot[0] % 3 != 0:
                nc.scalar.activation(
                    ex[:], qk2[:], mybir.ActivationFunctionType.Exp, scale=SCALE
                )
            else:
                nc.vector.tensor_scalar(
                    ex[:].bitcast(I8), qk2[:], SCH_A8, SCH_B8,
                    mybir.AluOpType.mult, mybir.AluOpType.add,
                )
            return ex

        av_tiles = {}

        def emit_av(p, P, ex):
            diag = P == p
            av = av_tiles[p]
            if diag:
                nc.tensor.matmul(
                    av[:], lhsT=v_sb[P][:, 0, :], rhs=ex[:, 0:QB],
                    start=(P == 0), stop=False,
                )
                nc.tensor.matmul(
                    av[:, O1:QB], lhsT=v_sb[P][:, 1, :],
                    rhs=ex[:, QB + O1 : 2 * QB], start=False, stop=True,
                )
            else:
                nc.tensor.matmul(
                    av_pad_tiles[p][:], lhsT=v8_sb[P][:, :, :],
                    rhs=ex[:].rearrange("p (two n) -> p two n", two=2),
                    start=(P == 0), stop=False, perf_mode=DR,
                )
            if diag:  # block finished: drain, DMA out
                ot = pout.tile([H + 1, QB], F32, tag="out")
                nc.vector.tensor_copy(ot[:], av[:])
                nc.sync.dma_start(out=ud[:, QB * p : QB * (p + 1)], in_=ot[:])

        av_pad_tiles = {}
        pending = []
        for p in range(NQB):
            av = pav.tile([H + 16, QB], F32, tag="av")
            av_pad_tiles[p] = av
            av_tiles[p] = av[0 : H + 1, :]
            for P in range(p + 1):
                ex = emit_qk_exp(p, P)
                pending.append((p, P, ex))
                if len(pending) > 2:
                    emit_av(*pending.pop(0))
        while pending:
            emit_av(*pending.pop(0))

    nc.compile()
    return nc


def get_nc() -> bass.Bass:
    global _NC
    if _NC is None:
        _NC = build_nc()
    return _NC


def make_in_maps(x, Wk, Wq, Wv):
    import ml_dtypes

    bf16 = ml_dtypes.bfloat16
    x = np.asarray(x, np.float32)

    def wpack(W):
        return np.ascontiguousarray(
            np.asarray(W, np.float32).reshape(2, 128, H).transpose(1, 0, 2)
        ).astype(bf16)

    wq8, wk8, wv8 = wpack(Wq), wpack(Wk), wpack(Wv)

    kk = np.arange(KT)[:, None]
    jj = np.arange(QB)[None, :]
    in_maps = []
    for core in range(8):
        b, par = divmod(core, 2)
        xb = x[b].T.reshape(2, 128, T).transpose(1, 0, 2)  # [128, 2, T]
        xq = np.ascontiguousarray(xb).astype(bf16)
        # gathered parity columns: local tile s -> global tile g=2s+par
        cols = (
            (2 * np.arange(TK // KT)[:, None] + par) * KT + np.arange(KT)[None, :]
        ).reshape(-1)
        xkv = np.ascontiguousarray(xb[:, :, cols]).astype(bf16)
        # mask [128, 512]: cols 0:256 for diag tile d0 (offset 128*par),
        # cols 256:512 for diag tile d1 (offset 256+128*par), both relative
        # to the computed regions (h0 cols 0:256 of q-block, h1 cols 256:512).
        m = np.zeros((KT, QB), np.float32)
        m[:, 0:O1] = (jj[:, 0:O1] >= kk + 128 * par).astype(np.float32)
        m[:, O1:QB] = (jj[:, O1:QB] >= kk + O1 + 128 * par).astype(np.float32)
        in_maps.append(
            {"xq": xq, "xkv": xkv, "wq": wq8, "wk": wk8, "wv": wv8,
             "msk": m.astype(bf16)}
        )
    return in_maps


def merge(results):
    out = np.empty((B, T, H), np.float32)
    for b in range(B):
        s = results[2 * b]["ud"] + results[2 * b + 1]["ud"]  # [65, T]
        out[b] = (s[0:H] / s[H : H + 1]).T
    return out


def kernel(x, Wk, Wq, Wv, **kw):
    in_maps = make_in_maps(x, Wk, Wq, Wv)
    res = run_bass_kernel_spmd(get_nc(), in_maps, core_ids=list(range(8)), **kw)
    out = merge(res.results)
    if kw:
        return out, res
    return out


# revision 26
# speedup vs baseline: 1.3428x; 1.3428x over previous
"""Causal single-head attention forward (B=4, T=4096, C=256, H=64) on 8 NeuronCores.

Sharding: core = (batch, kv_parity).  Each core processes ALL queries of its
batch but only kv tiles (128 keys) whose global tile index has its parity
(even/odd interleave), which balances the causal workload across the two
cores of a batch.  Each core emits unnormalized numerator+denominator stacked
as ud[65, T] (rows 0:64 = (exp(S)@V)^T, row 64 = sum exp(S)); the host
merges: out = (u0+u1)/(d0+d1), transposed back.

The compiled program is parity-uniform; parity only enters through host-
prepared data (gathered xkv columns and the diagonal-pair mask values).

Engine plan per core:
- PE (bf16): q projections from full x, k/v projections from the gathered
  parity columns, S^T = K Q^T per 128-key tile, AV with V stationary (ones
  column folds the denominator).  The diagonal pair's second tile is
  col-trimmed to [256, 512) (parity-uniform superset of the causal region).
  AV lags the QK/exp stream by 2 pairs so the PE queue never drains (keeps
  the tensor engine's p-state ramped).
- ACT: exact exp (fp32 psum -> bf16) for the diagonal pair + even non-diag
  pairs; table pre-warmed during input DMA.
- DVE: exp for odd non-diag pairs via one tensor_scalar emitting bf16 BITS
  as int16 (Schraudolph: bits = round(s*A + B), ~+-3% per-element, max-norm
  safe), q/k psum->bf16 casts, diagonal mask multiplies.
- Pool (gpsimd): v psum->bf16 copies, ones-column memsets, av psum->sbuf
  output copies.
"""

import sys

for _p in ("/opt/trn_rl_repo", "/root/.axon_site/_ro/trn_rl_repo"):
    if _p not in sys.path:
        sys.path.append(_p)

from contextlib import ExitStack

import numpy as np

import concourse.bacc as bacc
import concourse.bass as bass
import concourse.tile as tile
from concourse import mybir
from concourse.bass_utils import run_bass_kernel_spmd

B, T, C, H = 4, 4096, 256, 64
QB = 512         # query block width
NQB = T // QB    # 8 query blocks
KT = 128         # kv tile width
TK = T // 2      # gathered kv columns per core
F32 = mybir.dt.float32
BF16 = mybir.dt.bfloat16
FP8 = mybir.dt.float8e4
I16 = mybir.dt.int16
I8 = mybir.dt.int8
DR = mybir.MatmulPerfMode.DoubleRow
SCALE = float(C) ** -0.5
# Schraudolph: bits = round(s*A + B) ~ exp(s*SCALE), emitted as raw bf16/fp8 bits
SCH_A = 128.0 / float(np.log(2.0)) * SCALE
SCH_B = 16248.65
SCH_A8 = 8.0 / float(np.log(2.0)) * SCALE
SCH_B8 = 55.55
O1 = 256         # uniform col-trim offset for the diagonal pair's 2nd tile

_NC = None


def build_nc() -> bass.Bass:
    nc = bacc.Bacc("TRN2", target_bir_lowering=False, debug=False)
    xq = nc.declare_dram_parameter("xq", [128, 2, T], BF16, isOutput=False)
    xkv = nc.declare_dram_parameter("xkv", [128, 2, TK], BF16, isOutput=False)
    wq = nc.declare_dram_parameter("wq", [128, 2, H], BF16, isOutput=False)
    wk = nc.declare_dram_parameter("wk", [128, 2, H], BF16, isOutput=False)
    wv = nc.declare_dram_parameter("wv", [128, 2, H], BF16, isOutput=False)
    msk = nc.declare_dram_parameter("msk", [KT, QB], BF16, isOutput=False)
    ud = nc.declare_dram_parameter("ud", [H + 1, T], F32, isOutput=True)

    with tile.TileContext(nc) as tc, ExitStack() as ctx:
        persist = ctx.enter_context(tc.tile_pool(name="persist", bufs=1))
        pexp = ctx.enter_context(tc.tile_pool(name="exp", bufs=4))
        pout = ctx.enter_context(tc.tile_pool(name="out", bufs=2))
        pproj = ctx.enter_context(tc.tile_pool(name="pproj", bufs=2, space="PSUM"))
        pqk = ctx.enter_context(tc.tile_pool(name="pqk", bufs=2, space="PSUM"))
        pav = ctx.enter_context(tc.tile_pool(name="pav", bufs=2, space="PSUM"))

        # ---- input DMAs: need-first chunk order, split across SP+ACT queues --
        xkv_sb = persist.tile([128, 2, TK], BF16, tag="xkv")
        xq_sb = persist.tile([128, 2, T], BF16, tag="xq")
        nc.sync.dma_start(out=xkv_sb[:, :, 0:QB], in_=xkv[:, :, 0:QB])
        nc.scalar.dma_start(out=xq_sb[:, :, 0:QB], in_=xq[:, :, 0:QB])
        w_sb = {}
        for name, dram in (("k", wk), ("v", wv), ("q", wq)):
            t = persist.tile([128, 2, H], BF16, tag=f"w{name}")
            nc.sync.dma_start(out=t[:], in_=dram[:])
            w_sb[name] = t
        m_sb = persist.tile([KT, QB], BF16, tag="mask")
        nc.sync.dma_start(out=m_sb[:], in_=msk[:])
        # warm the ACT exp table while DMAs stream
        warm = persist.tile([1, 2], F32, tag="warm")
        nc.vector.memset(warm[:], 0.0)
        nc.scalar.activation(warm[:], warm[:], mybir.ActivationFunctionType.Exp)
        # remaining x chunks, interleaved need-first
        for j in range(1, NQB):
            nc.scalar.dma_start(
                out=xq_sb[:, :, QB * j : QB * (j + 1)],
                in_=xq[:, :, QB * j : QB * (j + 1)],
            )
            if j < TK // QB:
                nc.sync.dma_start(
                    out=xkv_sb[:, :, QB * j : QB * (j + 1)],
                    in_=xkv[:, :, QB * j : QB * (j + 1)],
                )

        # ---- projections (bf16, contract C in 2 chunks) ---------------------
        q_sb = [None] * NQB           # bf16 [64, QB]
        k_sb = [None] * (TK // QB)    # bf16 [64, QB] local gathered layout
        v_sb = [None] * NQB           # bf16 [128, 2, 65] per pair

        def proj_qk(which, src, j, dst_list):
            ps = pproj.tile([64, QB], F32, tag="proj")
            for c in range(2):
                nc.tensor.matmul(
                    ps[:], lhsT=w_sb[which][:, c, :],
                    rhs=src[:, c, QB * j : QB * (j + 1)],
                    start=(c == 0), stop=(c == 1),
                )
            t = persist.tile([64, QB], BF16, tag=f"{which}{j}")
            nc.vector.tensor_copy(t[:], ps[:])
            dst_list[j] = t

        def proj_v(P):
            # pair P covers local kv tiles 2P, 2P+1 -> gathered cols 128s
            ps = pproj.tile([128, 2, H], F32, tag="proj")
            for h in range(2):
                s = 2 * P + h
                for c in range(2):
                    nc.tensor.matmul(
                        ps[:, h, :],
                        lhsT=xkv_sb[:, c, KT * s : KT * (s + 1)],
                        rhs=w_sb["v"][:, c, :],
                        start=(c == 0), stop=(c == 1),
                    )
            t = persist.tile([128, 2, H + 1], BF16, tag=f"v{P}")
            nc.vector.tensor_copy(t[:, :, 0:H], ps[:])
            nc.gpsimd.memset(t[:, :, H : H + 1], 1.0)
            v_sb[P] = t

        # k/v first (block 0's deps), then interleave q
        for j in range(TK // QB):
            proj_qk("k", xkv_sb, j, k_sb)
            proj_v(2 * j)
            proj_v(2 * j + 1)
            proj_qk("q", xq_sb, 2 * j, q_sb)
            proj_qk("q", xq_sb, 2 * j + 1, q_sb)

        # ---- attention -------------------------------------------------------
        def k_slice(s):  # local kv tile s -> gathered k columns
            return k_sb[s // 4][:, KT * (s % 4) : KT * (s % 4 + 1)]

        nslot = [0]

        def emit_qk_exp(p, P):
            diag = P == p
            if not diag:
                nslot[0] += 1
            qk2 = pqk.tile([KT, 2 * QB], F32, tag="qk")
            nc.tensor.matmul(
                qk2[:, 0:QB], lhsT=k_slice(2 * P), rhs=q_sb[p][:],
                start=True, stop=True,
            )
            if diag:
                nc.tensor.matmul(
                    qk2[:, QB + O1 : 2 * QB], lhsT=k_slice(2 * P + 1),
                    rhs=q_sb[p][:, O1:QB], start=True, stop=True,
                )
            else:
                nc.tensor.matmul(
                    qk2[:, QB : 2 * QB], lhsT=k_slice(2 * P + 1), rhs=q_sb[p][:],
                    start=True, stop=True,
                )
            if diag:
                ex = pexp.tile([KT, 2 * QB], BF16, tag="exp")
                nc.scalar.activation(
                    ex[:, 0:QB], qk2[:, 0:QB],
                    mybir.ActivationFunctionType.Exp, scale=SCALE,
                )
                nc.scalar.activation(
                    ex[:, QB + O1 : 2 * QB], qk2[:, QB + O1 : 2 * QB],
                    mybir.ActivationFunctionType.Exp, scale=SCALE,
                )
                # masks: region h0 = ex[:, 0:256] (*= msk[:, 0:256]),
                #        region h1 = ex[:, 768:1024] (*= msk[:, 256:512])
                nc.gpsimd.tensor_mul(ex[:, 0:O1], ex[:, 0:O1], m_sb[:, 0:O1])
                nc.gpsimd.tensor_mul(
                    ex[:, QB + O1 : 2 * QB], ex[:, QB + O1 : 2 * QB],
                    m_sb[:, O1:QB],
                )
                return ex
            ex = pexp.tile([KT, 2 * QB], BF16, tag="exp")
            if nslot[0] % 3 != 0:
                nc.scalar.activation(
                    ex[:], qk2[:], mybir.ActivationFunctionType.Exp, scale=SCALE
                )
            else:
                nc.vector.tensor_scalar(
                    ex[:].bitcast(I16), qk2[:], SCH_A, SCH_B,
                    mybir.AluOpType.mult, mybir.AluOpType.add,
                )
            return ex

        av_tiles = {}

        def emit_av(p, P, ex):
            diag = P == p
            av = av_tiles[p]
            if diag:
                nc.tensor.matmul(
                    av[:], lhsT=v_sb[P][:, 0, :], rhs=ex[:, 0:QB],
                    start=(P == 0), stop=False,
                )
                nc.tensor.matmul(
                    av[:, O1:QB], lhsT=v_sb[P][:, 1, :],
                    rhs=ex[:, QB + O1 : 2 * QB], start=False, stop=True,
                )
            else:
                nc.tensor.matmul(
                    av[:], lhsT=v_sb[P][:, 0, :], rhs=ex[:, 0:QB],
                    start=(P == 0), stop=False,
                )
                nc.tensor.matmul(
                    av[:, 0:QB], lhsT=v_sb[P][:, 1, :], rhs=ex[:, QB : 2 * QB],
                    start=False, stop=False,
                )
            if diag:  # block finished: drain, DMA out
                ot = pout.tile([H + 1, QB], F32, tag="out")
                nc.vector.tensor_copy(ot[:], av[:])
                nc.sync.dma_start(out=ud[:, QB * p : QB * (p + 1)], in_=ot[:])

        pending = []
        for p in range(NQB):
            av = pav.tile([H + 1, QB], F32, tag="av")
            av_tiles[p] = av
            for P in range(p + 1):
                ex = emit_qk_exp(p, P)
                pending.append((p, P, ex))
                if len(pending) > 2:
                    emit_av(*pending.pop(0))
        while pending:
            emit_av(*pending.pop(0))

    nc.compile()
    return nc


def get_nc() -> bass.Bass:
    global _NC
    if _NC is None:
        _NC = build_nc()
    return _NC


def make_in_maps(x, Wk, Wq, Wv):
    import ml_dtypes

    bf16 = ml_dtypes.bfloat16
    x = np.asarray(x, np.float32)

    def wpack(W):
        return np.ascontiguousarray(
            np.asarray(W, np.float32).reshape(2, 128, H).transpose(1, 0, 2)
        ).astype(bf16)

    wq8, wk8, wv8 = wpack(Wq), wpack(Wk), wpack(Wv)

    kk = np.arange(KT)[:, None]
    jj = np.arange(QB)[None, :]
    in_maps = []
    for core in range(8):
        b, par = divmod(core, 2)
        xb = x[b].T.reshape(2, 128, T).transpose(1, 0, 2)  # [128, 2, T]
        xq = np.ascontiguousarray(xb).astype(bf16)
        # gathered parity columns: local tile s -> global tile g=2s+par
        cols = (
            (2 * np.arange(TK // KT)[:, None] + par) * KT + np.arange(KT)[None, :]
        ).reshape(-1)
        xkv = np.ascontiguousarray(xb[:, :, cols]).astype(bf16)
        # mask [128, 512]: cols 0:256 for diag tile d0 (offset 128*par),
        # cols 256:512 for diag tile d1 (offset 256+128*par), both relative
        # to the computed regions (h0 cols 0:256 of q-block, h1 cols 256:512).
        m = np.zeros((KT, QB), np.float32)
        m[:, 0:O1] = (jj[:, 0:O1] >= kk + 128 * par).astype(np.float32)
        m[:, O1:QB] = (jj[:, O1:QB] >= kk + O1 + 128 * par).astype(np.float32)
        in_maps.append(
            {"xq": xq, "xkv": xkv, "wq": wq8, "wk": wk8, "wv": wv8,
             "msk": m.astype(bf16)}
        )
    return in_maps


def merge(results):
    out = np.empty((B, T, H), np.float32)
    for b in range(B):
        s = results[2 * b]["ud"] + results[2 * b + 1]["ud"]  # [65, T]
        out[b] = (s[0:H] / s[H : H + 1]).T
    return out


def kernel(x, Wk, Wq, Wv, **kw):
    in_maps = make_in_maps(x, Wk, Wq, Wv)
    res = run_bass_kernel_spmd(get_nc(), in_maps, core_ids=list(range(8)), **kw)
    out = merge(res.results)
    if kw:
        return out, res
    return out
